# revision 17
# baseline (speedup 1.0000x reference)
"""Grouped-GEMM (MoE routing) kernel for TRN2, 8 NeuronCores, SPMD.

out[m] = values[m] @ combining_matrix[species_idx[m]]
  values [131072, 128] f32, species_idx [131072] i32, combining_matrix [8, 128, 256] f32

Strategy (v4 — species-parallel, fp8 input, drain engines freed of DMA):
  - Host: route rows by species; core c gets ALL rows of species c
    (counts are 16384 +-1%, so cores stay balanced). Each core's rows are
    packed into a transposed buffer xT [128, R] (zero-padded to the max
    species count R, identical on every core -> one SPMD program).
  - Values travel as fp8 e3m4 (1 B/elem): the PE consumes the fp8 moving
    operand against bf16 stationary weights directly (both upcast to fp22
    internally; fp8 runs at bf16 speed). Measured end-to-end max rel
    error 1.37e-2 vs the 2e-2 gate (bf16 path: 3.2e-3).
  - Per-core HBM: 2.1 MB x + 64 KB w + 8.5 MB out = 10.7 MB -> ~29 us at
    the ~370 GB/s per-core HBM limit; that DMA stream is the roofline.
  - Drain is the scarce engine resource (PSUM reads are 1 elem/cycle/lane
    on DVE@0.96GHz / ACT@1.2GHz, fp32 source; GpSimd/DMA have no PSUM
    port). Per 1024-col group, h=0 drains on DVE out of psA and h=1 on
    ACT out of psB concurrently; each half is 2-deep in PSUM (4x2 banks)
    so the group cadence stays at one cast (~1.4 us) < DMA pace 1.77 us.
  - Cast engines never issue DMAs (that cost ~0.7 us each and made ACT
    the critical path): h=0 flushes ride the SP HWDGE ring, h=1 the
    GpSimd SWDGE ring, one 0.26 MB piece per group.
  - Host: scatter outT columns back to the full [131072, 256] f32 output.
"""

import numpy as np
import ml_dtypes
from contextlib import ExitStack

import concourse.bass as bass
import concourse.mybir as mybir
import concourse.tile as tile
from concourse import bacc
from concourse.bass_utils import run_bass_kernel_spmd

M_TOTAL = 131072
D_IN = 128
N_OUT = 256
N_SPECIES = 8
N_CORES = 8
PAD = 16           # column padding granularity (rows of the sample axis)
CHUNK = 512        # matmul moving-dim chunk (PSUM bank limit)
GROUP = 1024       # sample cols per PSUM group (x2 halves = 4 banks)
SEG = 2048         # input DMA segment (cols)
F32 = mybir.dt.float32
BF16 = ml_dtypes.bfloat16
FP8 = ml_dtypes.float8_e3m4
X_DT = mybir.dt.float8e3
MM_DT = mybir.dt.bfloat16
OUT_DT = mybir.dt.bfloat16


def _segments(r_pad):
    """Input DMA segments with escalating sizes [1024, 2048, 4096, 4096,
    rest]: small early segments so compute ramps with low latency, big late
    ones so the input stream is not issue-paced (each DMA issue costs
    ~0.7-1 us on the SP ring). Every segment is a multiple of GROUP except
    the last, so 1024-col groups never straddle one."""
    sizes = [GROUP, 2 * GROUP, 4 * GROUP, 4 * GROUP]
    segs = []
    pos = 0
    for sz in sizes:
        if r_pad - pos <= 0:
            break
        sl = min(sz, r_pad - pos)
        segs.append((pos, sl))
        pos += sl
    while r_pad - pos > 0:        # one big tail segment (plus overflow)
        sl = min(8 * GROUP, r_pad - pos)
        segs.append((pos, sl))
        pos += sl
    return segs


def _groups(segs):
    groups = []
    for s0, sl in segs:
        g0 = 0
        while g0 < sl:
            gw = min(GROUP, sl - g0)
            groups.append((s0, g0, gw))
            g0 += gw
    return groups


def _pieces(r_pad):
    """Output flush pieces [(col_off, cols)] in groups-per-piece counts
    [1, 2, 3, 3, ..., 1, rem]: small first pieces so the output stream
    starts right behind the input stream, big middle pieces so each DMA
    ring stays at <=8 outstanding DMAs (the Tile scheduler has 8
    completion semaphores per ring — DMA #k's issue waits #k-8's
    completion, so long queues serialize the kernel tail), and small
    final pieces so the tail after the last cast is short."""
    groups = _groups(_segments(r_pad))
    n = len(groups)
    plan_sizes = [1, 2] + [3] * max(0, (n - 5) // 3 + 1)
    pieces = []
    gi = 0
    for sz in plan_sizes:
        if gi >= n - 2:
            break
        sz = min(sz, n - 2 - gi)
        if sz <= 0:
            break
        off = groups[gi][0] + groups[gi][1]
        end_g = groups[gi + sz - 1]
        end = end_g[0] + end_g[1] + end_g[2]
        pieces.append((off, end - off))
        gi += sz
    while gi < n:                 # final two groups flushed singly
        s0, g0, gw = groups[gi]
        pieces.append((s0 + g0, gw))
        gi += 1
    return pieces


def _build_nc(r_pad):
    """Build the SPMD program for one core; r_pad = padded max species count."""
    nc = bacc.Bacc("TRN2", target_bir_lowering=False, debug=False,
                   num_devices=N_CORES)
    xT = nc.dram_tensor("xT", [D_IN, r_pad], X_DT, kind="ExternalInput").ap()
    w = nc.dram_tensor("w", [D_IN, N_OUT], MM_DT, kind="ExternalInput").ap()
    # tiled output layout: piece (off, C) of half h lives at
    # outT[h, off*128 : (off+C)*128] as a contiguous [128, C] block, so
    # every out-DMA writes one sequential DRAM extent (HBM-friendly)
    # instead of 128 rows strided r_pad*2 bytes apart.
    outT = nc.dram_tensor("outT", [2, r_pad * 128], OUT_DT,
                          kind="ExternalOutput").ap()

    segs = _segments(r_pad)

    with tile.TileContext(nc) as tc, ExitStack() as ctx:
        wpool = ctx.enter_context(tc.tile_pool(name="w", bufs=1))
        xpool = ctx.enter_context(tc.tile_pool(name="x", bufs=len(segs)))
        opool = ctx.enter_context(tc.tile_pool(name="o", bufs=2))
        psA = ctx.enter_context(tc.tile_pool(name="psA", bufs=2, space="PSUM"))
        psB = ctx.enter_context(tc.tile_pool(name="psB", bufs=2, space="PSUM"))

        # weights ride the otherwise-idle ACT ring concurrently with the
        # input stream on the SP ring (one DMA per segment, all up-front).
        wt = wpool.tile([D_IN, N_OUT], MM_DT)
        nc.scalar.dma_start(wt, w)

        xts = {}
        for s0, sl in segs:
            xt = xpool.tile([D_IN, sl], X_DT, tag="x", name=f"x{s0}")
            xts[s0] = xt
            nc.sync.dma_start(xt, xT[:, s0:s0 + sl])

        ots = [opool.tile([128, r_pad], OUT_DT, tag="o", name=f"ot{h}")
               for h in range(2)]

        groups = _groups(segs)
        flush_at = {}                 # last group index of each piece
        for off, cols in _pieces(r_pad):
            end = off + cols
            for gi, (s0, g0, gw) in enumerate(groups):
                if s0 + g0 + gw == end:
                    flush_at[gi] = (off, cols)

        # per group: h=0 drains on DVE out of psA, h=1 on ACT out of psB —
        # the two 1024-col casts run concurrently and each half has a
        # 2-deep PSUM pipeline, so the group cadence stays at the cast
        # duration (~1.4 us) and under the DMA pace (~1.77 us/group).
        # Cast engines never issue DMAs (that would make them the critical
        # path); all flushes ride the SP HWDGE ring.
        pools = [psA, psB]
        for gi, (s0, g0, gw) in enumerate(groups):
            xseg = xts[s0]
            a0 = s0 + g0              # absolute output column
            for h in range(2):
                lhsT = wt[:, h * 128:(h + 1) * 128]
                ps = pools[h].tile([128, GROUP], F32, tag="ps",
                                   name=f"ps{h}g{gi}")
                for j in range(0, gw, CHUNK):
                    cj = min(CHUNK, gw - j)
                    nc.tensor.matmul(ps[:, j:j + cj], lhsT,
                                     xseg[:, g0 + j:g0 + j + cj],
                                     start=True, stop=True)
                if h == 0:
                    nc.vector.tensor_copy(ots[0][:, a0:a0 + gw], ps[:, :gw])
                else:
                    nc.scalar.copy(ots[1][:, a0:a0 + gw], ps[:, :gw])
            if gi in flush_at:
                off, cols = flush_at[gi]
                # both halves flush on the SP HWDGE ring (measured faster
                # than the GpSimd SWDGE ring); the piece plan keeps every
                # DMA's 8-back completion-sem recycle dependency on an
                # early, already-landed DMA. The final two pieces split
                # h1 onto the ACT ring — ACT's queue is free after its
                # last cast, and late SP issues inflate to ~1.3 us under
                # ring backpressure, so parallelizing them trims the tail.
                last2 = gi >= len(groups) - 2
                for h in range(2):
                    deng = nc.scalar if (h == 1 and last2) else nc.sync
                    deng.dma_start(
                        outT[h:h + 1, off * 128:(off + cols) * 128],
                        ots[h][:, off:off + cols])

    nc.compile()
    return nc


def _prepare(values, species_idx, combining_matrix):
    """Host routing + packing. Returns (in_maps, plan)."""
    values = np.ascontiguousarray(values, dtype=np.float32)
    species_idx = np.asarray(species_idx, dtype=np.int32)
    w = np.asarray(combining_matrix, dtype=np.float32)

    rows = [np.nonzero(species_idx == c)[0] for c in range(N_CORES)]
    counts = [r.size for r in rows]
    r_pad = -(-max(max(counts), GROUP) // PAD) * PAD

    in_maps = []
    for c in range(N_CORES):
        xT = np.zeros((D_IN, r_pad), dtype=FP8)
        n = counts[c]
        if n:
            xT[:, :n] = values[rows[c]].astype(FP8).T
        in_maps.append({"xT": xT, "w": np.ascontiguousarray(w[c].astype(BF16))})

    plan = {"rows": rows, "counts": counts, "r_pad": r_pad}
    return in_maps, plan


def _postprocess(results, plan):
    rows, counts, r_pad = plan["rows"], plan["counts"], plan["r_pad"]
    pieces = _pieces(r_pad)
    out = np.empty((M_TOTAL, N_OUT), dtype=np.float32)
    for c in range(N_CORES):
        n = counts[c]
        if not n:
            continue
        oT = results[c]["outT"]       # [2, r_pad*128] tiled bf16
        full = np.empty((N_OUT, r_pad), dtype=np.float32)
        for h in range(2):
            for off, cols in pieces:
                full[h * 128:(h + 1) * 128, off:off + cols] = \
                    oT[h, off * 128:(off + cols) * 128].reshape(128, cols)
        out[rows[c]] = full[:, :n].T
    return out


def kernel(values, species_idx, combining_matrix):
    in_maps, plan = _prepare(values, species_idx, combining_matrix)
    nc = _build_nc(plan["r_pad"])
    res = run_bass_kernel_spmd(nc, in_maps, list(range(N_CORES)))
    return _postprocess(res.results, plan)


# revision 19
# speedup vs baseline: 1.1936x; 1.1936x over previous
"""Grouped-GEMM (MoE routing) kernel for TRN2, 8 NeuronCores, SPMD.

out[m] = values[m] @ combining_matrix[species_idx[m]]
  values [131072, 128] f32, species_idx [131072] i32, combining_matrix [8, 128, 256] f32

Strategy (v4 — species-parallel, fp8 input, drain engines freed of DMA):
  - Host: route rows by species; core c gets ALL rows of species c
    (counts are 16384 +-1%, so cores stay balanced). Each core's rows are
    packed into a transposed buffer xT [128, R] (zero-padded to the max
    species count R, identical on every core -> one SPMD program).
  - Values travel as fp8 e3m4 (1 B/elem): the PE consumes the fp8 moving
    operand against bf16 stationary weights directly (both upcast to fp22
    internally; fp8 runs at bf16 speed). Measured end-to-end max rel
    error 1.37e-2 vs the 2e-2 gate (bf16 path: 3.2e-3).
  - Per-core HBM: 2.1 MB x + 64 KB w + 8.5 MB out = 10.7 MB -> ~29 us at
    the ~370 GB/s per-core HBM limit; that DMA stream is the roofline.
  - Drain is the scarce engine resource (PSUM reads are 1 elem/cycle/lane
    on DVE@0.96GHz / ACT@1.2GHz, fp32 source; GpSimd/DMA have no PSUM
    port). Per 1024-col group, h=0 drains on DVE out of psA and h=1 on
    ACT out of psB concurrently; each half is 2-deep in PSUM (4x2 banks)
    so the group cadence stays at one cast (~1.4 us) < DMA pace 1.77 us.
  - Cast engines never issue DMAs (that cost ~0.7 us each and made ACT
    the critical path): h=0 flushes ride the SP HWDGE ring, h=1 the
    GpSimd SWDGE ring, one 0.26 MB piece per group.
  - Host: scatter outT columns back to the full [131072, 256] f32 output.
"""

import numpy as np
import ml_dtypes
from contextlib import ExitStack

import concourse.bass as bass
import concourse.mybir as mybir
import concourse.tile as tile
from concourse import bacc
from concourse.bass_utils import run_bass_kernel_spmd

M_TOTAL = 131072
D_IN = 128
N_OUT = 256
N_SPECIES = 8
N_CORES = 8
PAD = 16           # column padding granularity (rows of the sample axis)
CHUNK = 512        # matmul moving-dim chunk (PSUM bank limit)
GROUP = 1024       # sample cols per PSUM group (x2 halves = 4 banks)
SEG = 2048         # input DMA segment (cols)
F32 = mybir.dt.float32
BF16 = ml_dtypes.bfloat16
FP8 = ml_dtypes.float8_e3m4
X_DT = mybir.dt.float8e3
MM_DT = mybir.dt.bfloat16
OUT_DT = mybir.dt.bfloat16


def _segments(r_pad):
    """Input DMA segments with escalating sizes [1024, 2048, 4096, 4096,
    rest]: small early segments so compute ramps with low latency, big late
    ones so the input stream is not issue-paced (each DMA issue costs
    ~0.7-1 us on the SP ring). Every segment is a multiple of GROUP except
    the last, so 1024-col groups never straddle one."""
    sizes = [GROUP, 2 * GROUP, 4 * GROUP, 4 * GROUP]
    segs = []
    pos = 0
    for sz in sizes:
        if r_pad - pos <= 0:
            break
        sl = min(sz, r_pad - pos)
        segs.append((pos, sl))
        pos += sl
    while r_pad - pos > 0:        # one big tail segment (plus overflow)
        sl = min(8 * GROUP, r_pad - pos)
        segs.append((pos, sl))
        pos += sl
    return segs


def _groups(segs):
    groups = []
    for s0, sl in segs:
        g0 = 0
        while g0 < sl:
            gw = min(GROUP, sl - g0)
            groups.append((s0, g0, gw))
            g0 += gw
    return groups


def _pieces(r_pad):
    """Output flush pieces [(col_off, cols)] in groups-per-piece counts
    [1, 2, 3, 3, ..., 2]: small first pieces so the output stream starts
    right behind the input stream, big middle pieces so the SP ring's
    8-deep completion-semaphore recycle (DMA #k's issue waits #k-8's
    completion) never stalls, and the final TWO groups merged into one
    piece — late SP issues cost ~1.3 us under ring backpressure, so one
    fewer serialized issue pair beats a smaller final transfer."""
    groups = _groups(_segments(r_pad))
    n = len(groups)
    plan_sizes = [1, 2] + [3] * max(0, (n - 5) // 3 + 1)
    pieces = []
    gi = 0
    for sz in plan_sizes:
        if gi >= n - 2:
            break
        sz = min(sz, n - 2 - gi)
        if sz <= 0:
            break
        off = groups[gi][0] + groups[gi][1]
        end_g = groups[gi + sz - 1]
        end = end_g[0] + end_g[1] + end_g[2]
        pieces.append((off, end - off))
        gi += sz
    if gi < n:                    # final piece: the remaining groups
        off = groups[gi][0] + groups[gi][1]
        pieces.append((off, r_pad - off))
    return pieces


def _build_nc(r_pad):
    """Build the SPMD program for one core; r_pad = padded max species count."""
    nc = bacc.Bacc("TRN2", target_bir_lowering=False, debug=False,
                   num_devices=N_CORES)
    xT = nc.dram_tensor("xT", [D_IN, r_pad], X_DT, kind="ExternalInput").ap()
    w = nc.dram_tensor("w", [D_IN, N_OUT], MM_DT, kind="ExternalInput").ap()
    # tiled output layout: piece (off, C) of half h lives at
    # outT[h, off*128 : (off+C)*128] as a contiguous [128, C] block, so
    # every out-DMA writes one sequential DRAM extent (HBM-friendly)
    # instead of 128 rows strided r_pad*2 bytes apart.
    outT = nc.dram_tensor("outT", [2, r_pad * 128], OUT_DT,
                          kind="ExternalOutput").ap()

    segs = _segments(r_pad)

    with tile.TileContext(nc) as tc, ExitStack() as ctx:
        wpool = ctx.enter_context(tc.tile_pool(name="w", bufs=1))
        xpool = ctx.enter_context(tc.tile_pool(name="x", bufs=len(segs)))
        opool = ctx.enter_context(tc.tile_pool(name="o", bufs=2))
        psA = ctx.enter_context(tc.tile_pool(name="psA", bufs=2, space="PSUM"))
        psB = ctx.enter_context(tc.tile_pool(name="psB", bufs=2, space="PSUM"))

        # weights ride the otherwise-idle ACT ring concurrently with the
        # input stream on the SP ring (one DMA per segment, all up-front).
        wt = wpool.tile([D_IN, N_OUT], MM_DT)
        nc.scalar.dma_start(wt, w)

        xts = {}
        for s0, sl in segs:
            xt = xpool.tile([D_IN, sl], X_DT, tag="x", name=f"x{s0}")
            xts[s0] = xt
            nc.sync.dma_start(xt, xT[:, s0:s0 + sl])

        ots = [opool.tile([128, r_pad], OUT_DT, tag="o", name=f"ot{h}")
               for h in range(2)]

        groups = _groups(segs)
        flush_at = {}                 # last group index of each piece
        for off, cols in _pieces(r_pad):
            end = off + cols
            for gi, (s0, g0, gw) in enumerate(groups):
                if s0 + g0 + gw == end:
                    flush_at[gi] = (off, cols)

        # per group: h=0 drains on DVE out of psA, h=1 on ACT out of psB —
        # the two 1024-col casts run concurrently and each half has a
        # 2-deep PSUM pipeline, so the group cadence stays at the cast
        # duration (~1.4 us) and under the DMA pace (~1.77 us/group).
        # Cast engines never issue DMAs (that would make them the critical
        # path); all flushes ride the SP HWDGE ring.
        pools = [psA, psB]
        for gi, (s0, g0, gw) in enumerate(groups):
            xseg = xts[s0]
            a0 = s0 + g0              # absolute output column
            for h in range(2):
                lhsT = wt[:, h * 128:(h + 1) * 128]
                ps = pools[h].tile([128, GROUP], F32, tag="ps",
                                   name=f"ps{h}g{gi}")
                for j in range(0, gw, CHUNK):
                    cj = min(CHUNK, gw - j)
                    nc.tensor.matmul(ps[:, j:j + cj], lhsT,
                                     xseg[:, g0 + j:g0 + j + cj],
                                     start=True, stop=True)
                if h == 0:
                    nc.vector.tensor_copy(ots[0][:, a0:a0 + gw], ps[:, :gw])
                else:
                    nc.scalar.copy(ots[1][:, a0:a0 + gw], ps[:, :gw])
            if gi in flush_at:
                off, cols = flush_at[gi]
                # both halves flush on the SP HWDGE ring (measured faster
                # than the GpSimd SWDGE ring, and routing any flush via
                # the ACT ring measured much slower); the piece plan
                # keeps every DMA's 8-back completion-sem recycle
                # dependency on an early, already-landed DMA.
                for h in range(2):
                    nc.sync.dma_start(
                        outT[h:h + 1, off * 128:(off + cols) * 128],
                        ots[h][:, off:off + cols])

    nc.compile()
    return nc


def _prepare(values, species_idx, combining_matrix):
    """Host routing + packing. Returns (in_maps, plan)."""
    values = np.ascontiguousarray(values, dtype=np.float32)
    species_idx = np.asarray(species_idx, dtype=np.int32)
    w = np.asarray(combining_matrix, dtype=np.float32)

    rows = [np.nonzero(species_idx == c)[0] for c in range(N_CORES)]
    counts = [r.size for r in rows]
    r_pad = -(-max(max(counts), GROUP) // PAD) * PAD

    in_maps = []
    for c in range(N_CORES):
        xT = np.zeros((D_IN, r_pad), dtype=FP8)
        n = counts[c]
        if n:
            xT[:, :n] = values[rows[c]].astype(FP8).T
        in_maps.append({"xT": xT, "w": np.ascontiguousarray(w[c].astype(BF16))})

    plan = {"rows": rows, "counts": counts, "r_pad": r_pad}
    return in_maps, plan


def _postprocess(results, plan):
    rows, counts, r_pad = plan["rows"], plan["counts"], plan["r_pad"]
    pieces = _pieces(r_pad)
    out = np.empty((M_TOTAL, N_OUT), dtype=np.float32)
    for c in range(N_CORES):
        n = counts[c]
        if not n:
            continue
        oT = results[c]["outT"]       # [2, r_pad*128] tiled bf16
        full = np.empty((N_OUT, r_pad), dtype=np.float32)
        for h in range(2):
            for off, cols in pieces:
                full[h * 128:(h + 1) * 128, off:off + cols] = \
                    oT[h, off * 128:(off + cols) * 128].reshape(128, cols)
        out[rows[c]] = full[:, :n].T
    return out


def kernel(values, species_idx, combining_matrix):
    in_maps, plan = _prepare(values, species_idx, combining_matrix)
    nc = _build_nc(plan["r_pad"])
    res = run_bass_kernel_spmd(nc, in_maps, list(range(N_CORES)))
    return _postprocess(res.results, plan)
